# revision 15
# baseline (speedup 1.0000x reference)
"""CapsuleNet Trainium2 kernel, v2.

Data-parallel over batch: 64 items -> 8 cores x 8 items (4 pairs).

Math per item (matches reference):
  e   = emb[x] * mask                      [L=512, E=512]
  h   = relu(conv1d(e.T, k=9, pad=4) + b1) [C=32, L=512]
  p   = conv1d(h, k=9, pad=4, stride=2)+b2 [UC=256, S=256]
  p   = squash(p over C-blocks)
  routing (R=3), b logits S-independent:
    c = softmax_k(b); s[k] = sum_u c[u,k] (W[u,k].T @ p_u)
    v[k] = squash_c(s[k]); b += <W[u,k], p_u.T @ v[k]>
  out = mean_s(v)                          [K=9, C=32]

v2 design vs v1 (508us):
  - bf16 data path for all big matmuls (emb table pre-cast on host).
  - conv1 via taps-on-M: 3 tap-groups {t: t%3==g} x 32ch = M=96 rows,
    psum accumulates over (g,ec) with rhs column shift g; only a 3-way
    j-collapse (cross-quadrant DVE adds) remains. 24 matmuls of N=262
    per item instead of 36 of N=512 (3x fewer PE rows).
  - items processed in PAIRS: routing tiles [m, 512] halve instruction
    counts; shared-lhsT matmuls run at N=512.
  - s/G psums read directly by DVE (no SBUF staging copies).
  - reciprocal_approx_fast + affine_mul_reduce for squash chains.
  - softmax exp batched globally (2 ACT table switches, not 32).
"""

import numpy as np

import concourse.bass as bass
import concourse.tile as tile
from concourse import bacc, mybir
from concourse.bass_utils import run_bass_kernel_spmd

F32 = mybir.dt.float32
F32R = mybir.dt.float32r
BF16 = mybir.dt.bfloat16
I32 = mybir.dt.int32
AF = mybir.ActivationFunctionType
AX = mybir.AxisListType

V, E, L = 50000, 512, 512
B, U, C, K, R = 64, 8, 32, 9, 3
S = 256
NCORES = 8
BL = B // NCORES

KGS = [(0, 128), (128, 128), (256, 32)]


def _emit(tc, nc, aps, bl):
    from contextlib import ExitStack

    es = ExitStack()
    np_pairs = bl // 2

    def MM(out, lhsT, rhs, **kw):
        return nc.tensor.matmul(out=out, lhsT=lhsT, rhs=rhs, **kw)

    def MMr(out, lhsT, rhs, **kw):
        return nc.tensor.matmul(
            out=out, lhsT=lhsT.bitcast(F32R), rhs=rhs.bitcast(F32R), **kw
        )

    cp = es.enter_context(tc.tile_pool(name="consts", bufs=1))

    def const(cname, shape, dt):
        t = cp.tile(shape, dt, name=cname)
        nc.sync.dma_start(out=t[:], in_=aps[cname])
        return t

    w1g = const("w1g", [128, 1152], BF16)
    w2b = const("w2b", [32, 2304], BF16)
    wf = const("wf", [128, 576], F32)
    w9b = const("w9b", [128, 576], BF16)
    sq8 = const("sq8", [128, 16], BF16)
    it8 = const("it8", [8, 256], F32R)
    kindb = const("kindb", [128, 36], BF16)
    kindT = const("kindT", [12, 288], F32R)
    uindb = const("uindb", [128, 4], BF16)
    u4Tb = const("u4Tb", [4, 128], BF16)
    identf = const("identf", [128, 128], F32)
    identb = const("identb", [128, 128], BF16)
    b1 = const("b1", [32, 1], F32)
    b2 = const("b2", [128, 2], F32)
    fb = const("fb", [128, 2], F32)  # col0=1e-8, col1=1.0

    gp = es.enter_context(tc.tile_pool(name="gather", bufs=1))
    wp = es.enter_context(tc.tile_pool(name="work", bufs=2))
    pq = es.enter_context(tc.tile_pool(name="persist", bufs=1))
    pp = es.enter_context(tc.tile_pool(name="psum", bufs=1, space="PSUM"))

    # ---- batched index/mask load ----
    idxs = gp.tile([128, bl * 4], I32, name="idxs")
    nc.sync.dma_start(
        out=idxs[:].rearrange("p (it lc) -> p it lc", lc=4),
        in_=aps["x"].rearrange("it (lc p) -> p it lc", p=128),
    )
    msks = gp.tile([128, bl * 4], F32, name="msks")
    nc.sync.dma_start(
        out=msks[:].rearrange("p (it lc) -> p it lc", lc=4),
        in_=aps["mask"].rearrange("it (lc p) -> p it lc", p=128),
    )

    ERAW = {}

    def gather(it):
        for lc in range(4):
            er = gp.tile([128, 512], BF16, name=f"er{it}_{lc}", tag=f"er{lc}", bufs=3)
            col = it * 4 + lc
            nc.gpsimd.indirect_dma_start(
                out=er[:],
                out_offset=None,
                in_=aps["emb"],
                in_offset=bass.IndirectOffsetOnAxis(ap=idxs[:, col : col + 1], axis=0),
            )
            ERAW[(it, lc)] = er

    ET = [None] * bl
    HP = [None] * np_pairs
    PST = [None] * np_pairs
    PT = [None] * np_pairs
    WCS = [None] * bl
    SPS = {}
    CRSB = [None]
    S2 = [None] * np_pairs
    FKT = [None] * np_pairs
    VSB = [None] * np_pairs

    BT = pq.tile([4, 18 * bl], F32, name="BT")
    OUTB = [
        pq.tile([m, bl], F32, name=f"outb{kg}") for kg, (c0, m) in enumerate(KGS)
    ]

    # ---- stage A1: mask-diag transposes -> eTall ----
    def a_tp(it):
        eT = pq.tile([128, 4 * 528], BF16, name=f"eT{it}")
        ET[it] = eT
        er = eT[:].rearrange("p (ec l) -> p ec l", ec=4)
        nc.vector.memset(er[:, :, 0:4], 0.0)
        nc.vector.memset(er[:, :, 516:520], 0.0)
        tps = [
            pp.tile([128, 1024], BF16, name=f"tp{it}_{half}", tag="acv", bufs=3)
            for half in range(2)
        ]
        for lc in range(4):
            dm = wp.tile([128, 128], BF16, name=f"dm{it}_{lc}", tag="dm", bufs=4)
            col = it * 4 + lc
            nc.vector.tensor_scalar_mul(
                out=dm[:], in0=identf[:], scalar1=msks[:, col : col + 1]
            )
            for ec in range(4):
                nc.tensor.transpose(
                    out=tps[ec // 2][:, (ec % 2) * 512 + lc * 128 :][:, 0:128],
                    in_=ERAW[(it, lc)][:, ec * 128 : (ec + 1) * 128],
                    identity=dm[:],
                )
        for ec in range(4):
            src = tps[ec // 2][:, (ec % 2) * 512 :][:, 0:512]
            dst = eT[:, ec * 528 + 4 : ec * 528 + 516]
            if ec % 2 == 0:
                nc.vector.tensor_copy(out=dst, in_=src)
            else:
                nc.scalar.copy(out=dst, in_=src)

    # ---- stage A2: conv1 (taps-on-M) + collapse + relu ----
    def a_conv(it):
        p, it2 = it // 2, it % 2
        if it2 == 0:
            hp = pq.tile([32, 1056], BF16, name=f"hp{p}")
            HP[p] = hp
            hr = hp[:].rearrange("q (i l) -> q i l", i=2)
            nc.vector.memset(hr[:, :, 0:4], 0.0)
            nc.vector.memset(hr[:, :, 516:520], 0.0)
        hp = HP[p]
        eT = ET[it]
        for h in range(2):
            z = pp.tile([96, 262], F32, name=f"cv{it}_{h}", tag="acv", bufs=3)
            cnt = 0
            for g in range(3):
                for ec in range(4):
                    MM(
                        out=z[:],
                        lhsT=w1g[:, (g * 4 + ec) * 96 : (g * 4 + ec + 1) * 96],
                        rhs=eT[:, ec * 528 + h * 256 + g : ec * 528 + h * 256 + g + 262],
                        start=(cnt == 0),
                        stop=(cnt == 11),
                    )
                    cnt += 1
            zb = wp.tile([32, 256], F32, name=f"zb{it}_{h}", tag="zb", bufs=2)
            nc.scalar.copy(out=zb[:], in_=z[32:64, 3:259])
            u = wp.tile([32, 256], F32, name=f"u{it}_{h}", tag="clps", bufs=2)
            nc.vector.tensor_add(out=u[:], in0=z[0:32, 0:256], in1=zb[:])
            hpre = wp.tile([32, 256], F32, name=f"hpre{it}_{h}", tag="hpre", bufs=2)
            nc.vector.tensor_add(out=hpre[:], in0=u[:], in1=z[64:96, 6:262])
            nc.scalar.activation(
                out=hp[:, it2 * 528 + 4 + h * 256 : it2 * 528 + 4 + h * 256 + 256],
                in_=hpre[:],
                func=AF.Relu,
                bias=b1[:, 0:1],
            )

    # ---- stage B: primary conv (pair), squash-p, ps_t, pT ----
    def b_prim(p):
        hp = HP[p]
        prs = []
        for h in range(2):
            pr = pp.tile([128, 512], F32, name=f"pr{p}_{h}", tag="x", bufs=2)
            for t in range(9):
                rhs = hp[:].rearrange("q (i l) -> q i l", i=2)[:, :, t : t + 512]
                rhs = rhs.rearrange("q i (s two) -> q i s two", two=2)[:, :, :, 0]
                MM(
                    out=pr[:],
                    lhsT=w2b[:, t * 256 + h * 128 : t * 256 + (h + 1) * 128],
                    rhs=rhs,
                    start=(t == 0),
                    stop=(t == 8),
                )
            prs.append(pr)
        psb, p2 = [], []
        for h in range(2):
            sb = wp.tile([128, 512], F32, name=f"psb{p}_{h}", tag=f"psb{h}", bufs=2)
            nc.scalar.activation(
                out=sb[:], in_=prs[h][:], func=AF.Identity, bias=b2[:, h : h + 1]
            )
            psb.append(sb)
            q = wp.tile([128, 512], BF16, name=f"p2{p}_{h}", tag=f"p2{h}", bufs=2)
            nc.scalar.square(out=q[:], in_=prs[h][:])
            p2.append(q)
        psq = pp.tile([8, 512], F32, name=f"psq{p}", tag="x", bufs=2)
        MM(out=psq[:], lhsT=sq8[:, 0:8], rhs=p2[0][:], start=True, stop=False)
        MM(out=psq[:], lhsT=sq8[:, 8:16], rhs=p2[1][:], start=False, stop=True)
        u1 = wp.tile([8, 512], F32, name=f"u1p{p}", tag="u1p", bufs=1)
        nc.scalar.activation(out=u1[:], in_=psq[:], func=AF.Sqrt, bias=fb[0:8, 0:1])
        t3 = wp.tile([8, 512], F32, name=f"t3p{p}", tag="t3p", bufs=1)
        scr = wp.tile([8, 1], F32, name=f"scrp{p}", tag="scrp", bufs=1)
        nc.vector.affine_mul_reduce(
            out=t3[:], accum_out=scr[:], in0=psq[:], in1=u1[:], scale=1.0, bias=1.0
        )
        t4 = wp.tile([8, 512], F32, name=f"t4p{p}", tag="u1p", bufs=1)
        nc.vector.reciprocal_approx_fast(out=t4[:], in_=t3[:])
        f8 = wp.tile([8, 512], F32, name=f"f8{p}", tag="f8", bufs=1)
        nc.vector.tensor_mul(out=f8[:].bitcast(F32R), in0=psq[:], in1=t4[:])
        pst = []
        for h in range(2):
            pfb = pp.tile([128, 512], F32, name=f"pfb{p}_{h}", tag="x", bufs=2)
            MMr(out=pfb[:], lhsT=it8[:, h * 128 : (h + 1) * 128], rhs=f8[:],
                start=True, stop=True)
            ps = pq.tile([128, 512], BF16, name=f"pst{p}_{h}")
            nc.vector.tensor_mul(out=ps[:], in0=psb[h][:], in1=pfb[:])
            pst.append(ps)
        PST[p] = pst
        ptp = pp.tile([128, 1024], BF16, name=f"ptp{p}", tag="x", bufs=2)
        for sc in range(2):
            for it2 in range(2):
                for h in range(2):
                    nc.tensor.transpose(
                        out=ptp[:, sc * 512 + (it2 * 2 + h) * 128 :][:, 0:128],
                        in_=pst[h][:, it2 * 256 + sc * 128 : it2 * 256 + sc * 128 + 128],
                        identity=identb[:],
                    )
        pT = []
        for sc in range(2):
            t = pq.tile([128, 512], BF16, name=f"pT{p}_{sc}")
            if sc == 0:
                nc.vector.tensor_copy(out=t[:], in_=ptp[:, 0:512])
            else:
                nc.scalar.copy(out=t[:], in_=ptp[:, 512:1024])
            pT.append(t)
        PT[p] = pT

    # ---- routing: s + |s|^2 + squash-s ----
    def r_s(p, r):
        pst = PST[p]
        sps = []
        for kg, (c0, m) in enumerate(KGS):
            sp = pp.tile([m, 512], F32, name=f"s{p}_{r}_{kg}", tag="s", bufs=3)
            if r == 0:
                for ch in range(2):
                    MM(
                        out=sp[:],
                        lhsT=w9b[:, ch * 288 + c0 : ch * 288 + c0 + m],
                        rhs=pst[ch][:],
                        start=(ch == 0),
                        stop=(ch == 1),
                    )
            else:
                for it2 in range(2):
                    for ch in range(2):
                        MM(
                            out=sp[:, it2 * 256 : (it2 + 1) * 256],
                            lhsT=WCS[p * 2 + it2][ch][:, c0 : c0 + m],
                            rhs=pst[ch][:, it2 * 256 : (it2 + 1) * 256],
                            start=(ch == 0),
                            stop=(ch == 1),
                        )
            sps.append(sp)
        ssb = []
        for kg, (c0, m) in enumerate(KGS):
            sb = wp.tile([m, 512], BF16, name=f"ssb{p}_{r}_{kg}", tag=f"sb{kg}", bufs=4)
            if kg == 1:
                nc.vector.tensor_copy(out=sb[:], in_=sps[kg][:])
            else:
                nc.scalar.copy(out=sb[:], in_=sps[kg][:])
            ssb.append(sb)
        SPS[p] = ssb
        s2 = []
        for kg, (c0, m) in enumerate(KGS):
            q = wp.tile([m, 512], BF16, name=f"s2_{p}_{r}_{kg}", tag=f"sq{kg}", bufs=4)
            if kg == 1:
                nc.vector.tensor_mul(out=q[:], in0=ssb[kg][:], in1=ssb[kg][:])
            else:
                nc.scalar.square(out=q[:], in_=sps[kg][:])
            s2.append(q)
        S2[p] = s2
        qk = pp.tile([12, 512], F32, name=f"qk{p}_{r}", tag="x", bufs=2)
        for kg, (c0, m) in enumerate(KGS):
            MM(
                out=qk[:],
                lhsT=kindb[0:m, kg * 12 : (kg + 1) * 12],
                rhs=s2[kg][:],
                start=(kg == 0),
                stop=(kg == 2),
            )
        u1 = wp.tile([12, 512], F32, name=f"u1k{p}_{r}", tag="u1k", bufs=1)
        nc.scalar.activation(out=u1[:], in_=qk[:], func=AF.Sqrt, bias=fb[0:12, 0:1])
        sc_ = float(S) if r == R - 1 else 1.0
        t3 = wp.tile([12, 512], F32, name=f"t3k{p}_{r}", tag="t3k", bufs=1)
        scr = wp.tile([12, 1], F32, name=f"scrk{p}_{r}", tag="scrk", bufs=2)
        nc.vector.affine_mul_reduce(
            out=t3[:], accum_out=scr[:], in0=qk[:], in1=u1[:], scale=sc_, bias=sc_
        )
        t4 = wp.tile([12, 512], F32, name=f"t4k{p}_{r}", tag="t4k", bufs=1)
        nc.vector.reciprocal_approx_fast(out=t4[:], in_=t3[:])
        fkt = wp.tile([12, 512], F32, name=f"fkt{p}_{r}", tag="fk", bufs=1)
        nc.vector.tensor_mul(out=fkt[:].bitcast(F32R), in0=qk[:], in1=t4[:])
        FKT[p] = fkt

    # ---- routing: v (+ transpose + agree for r<R-1, emit for last r) ----
    def r_v(p, r):
        sps = SPS[p]
        fkt = FKT[p]
        last = r == R - 1
        vs = []
        for kg, (c0, m) in enumerate(KGS):
            vfb = pp.tile([m, 512], F32, name=f"vfb{p}_{r}_{kg}", tag="x", bufs=2)
            MMr(out=vfb[:], lhsT=kindT[:, c0 : c0 + m], rhs=fkt[:],
                start=True, stop=True)
            if last:
                v = wp.tile([m, 512], F32, name=f"v{p}_{r}_{kg}", tag=f"vf{kg}", bufs=1)
            else:
                v = wp.tile([m, 512], BF16, name=f"v{p}_{r}_{kg}", tag=f"vb{kg}", bufs=4)
            nc.vector.tensor_mul(out=v[:], in0=sps[kg][:], in1=vfb[:])
            vs.append(v)
        VSB[p] = vs
        if last:
            for kg, (c0, m) in enumerate(KGS):
                nc.vector.reduce_sum(
                    out=OUTB[kg][:, p * 2 : p * 2 + 2],
                    in_=vs[kg][:].rearrange("q (i s) -> q i s", i=2),
                    axis=AX.X,
                )
            return
        agr = wp.tile([4, 36], F32, name=f"agr{p}_{r}", tag="agr", bufs=4)
        for it2 in range(2):
            vtp = pp.tile([128, 576], BF16, name=f"vtp{p}_{r}_{it2}", tag="x", bufs=2)
            for sc in range(2):
                for kg, (c0, m) in enumerate(KGS):
                    nc.tensor.transpose(
                        out=vtp[:, sc * 288 + c0 :][:, 0:m],
                        in_=vs[kg][:, it2 * 256 + sc * 128 :][:, 0:128],
                        identity=identb[0:m, 0:m],
                    )
            vT = wp.tile([128, 576], BF16, name=f"vT{p}_{r}_{it2}", tag=f"vT{it2}", bufs=4)
            if it2 == 0:
                nc.vector.tensor_copy(out=vT[:], in_=vtp[:])
            else:
                nc.scalar.copy(out=vT[:], in_=vtp[:])
            for ch in range(2):
                gps = pp.tile([128, 288], F32, name=f"g{p}_{r}_{it2}_{ch}", tag="x", bufs=2)
                for sc in range(2):
                    MM(
                        out=gps[:],
                        lhsT=PT[p][sc][:, (it2 * 2 + ch) * 128 : (it2 * 2 + ch + 1) * 128],
                        rhs=vT[:, sc * 288 : (sc + 1) * 288],
                        start=(sc == 0),
                        stop=(sc == 1),
                    )
                ga = wp.tile([128, 288], BF16, name=f"ga{p}_{r}_{it2}_{ch}", tag="ga", bufs=3)
                nc.vector.tensor_mul(
                    out=ga[:], in0=wf[:, ch * 288 : (ch + 1) * 288], in1=gps[:]
                )
                ap_ = pp.tile([4, 288], F32, name=f"ap{p}_{r}_{it2}_{ch}", tag="x", bufs=2)
                MM(out=ap_[:], lhsT=uindb[:], rhs=ga[:], start=True, stop=True)
                nc.vector.reduce_sum(
                    out=agr[:, it2 * 18 + ch * 9 : it2 * 18 + ch * 9 + 9],
                    in_=ap_[:].rearrange("q (k c) -> q k c", c=32),
                    axis=AX.X,
                )
        bsl = BT[:, p * 36 : (p + 1) * 36]
        if r == 0:
            nc.vector.tensor_copy(out=bsl, in_=agr[:])
        else:
            nc.vector.tensor_add(out=bsl, in0=bsl, in1=agr[:])

    # ---- waved rounds r1/r2: all pairs per stage for engine-level overlap ----
    def round_wave(r):
        for grp in ([0], [1], [2], [3]) if np_pairs == 4 else ([list(range(np_pairs))]):
            round_wave_grp(r, grp)

    def round_wave_grp(r, grp):
        last = r == R - 1
        sps_all = {}
        for p in grp:
            build_wcs(p * 2)
            build_wcs(p * 2 + 1)
            pst = PST[p]
            sps = []
            for kg, (c0, m) in enumerate(KGS):
                sp = pp.tile([m, 512], F32, name=f"s{p}_{r}_{kg}", tag="s", bufs=3)
                for it2 in range(2):
                    for ch in range(2):
                        MM(
                            out=sp[:, it2 * 256 : (it2 + 1) * 256],
                            lhsT=WCS[p * 2 + it2][ch][:, c0 : c0 + m],
                            rhs=pst[ch][:, it2 * 256 : (it2 + 1) * 256],
                            start=(ch == 0),
                            stop=(ch == 1),
                        )
                sps.append(sp)
            sps_all[p] = sps
        ssb_all = {}
        for p in grp:
            ssb = []
            for kg, (c0, m) in enumerate(KGS):
                sb = wp.tile([m, 512], BF16, name=f"ssb{p}_{r}_{kg}", tag=f"sb{kg}", bufs=4)
                if kg == 1:
                    nc.vector.tensor_copy(out=sb[:], in_=sps_all[p][kg][:])
                else:
                    nc.scalar.copy(out=sb[:], in_=sps_all[p][kg][:])
                ssb.append(sb)
            ssb_all[p] = ssb
        s2_all = {}
        for p in grp:
            s2 = []
            for kg, (c0, m) in enumerate(KGS):
                q = wp.tile([m, 512], BF16, name=f"s2_{p}_{r}_{kg}", tag=f"sq{kg}", bufs=4)
                if kg == 1:
                    nc.vector.tensor_mul(out=q[:], in0=ssb_all[p][kg][:], in1=ssb_all[p][kg][:])
                else:
                    nc.scalar.square(out=q[:], in_=sps_all[p][kg][:])
                s2.append(q)
            s2_all[p] = s2
        sc_ = float(S) if last else 1.0
        fkt_all = {}
        for p in grp:
            qk = pp.tile([12, 512], F32, name=f"qk{p}_{r}", tag="x", bufs=2)
            for kg, (c0, m) in enumerate(KGS):
                MM(
                    out=qk[:],
                    lhsT=kindb[0:m, kg * 12 : (kg + 1) * 12],
                    rhs=s2_all[p][kg][:],
                    start=(kg == 0),
                    stop=(kg == 2),
                )
            qs = wp.tile([12, 512], F32, name=f"qs{p}_{r}", tag="qs", bufs=2)
            nc.vector.tensor_copy(out=qs[:], in_=qk[:])
            u1 = wp.tile([12, 512], F32, name=f"u1w{p}_{r}", tag="u1w", bufs=2)
            nc.scalar.activation(out=u1[:], in_=qk[:], func=AF.Sqrt, bias=fb[0:12, 0:1])
            t3 = wp.tile([12, 512], F32, name=f"t3w{p}_{r}", tag="t3w", bufs=2)
            scr = wp.tile([12, 1], F32, name=f"scrw{p}_{r}", tag="scrw", bufs=2)
            nc.vector.affine_mul_reduce(
                out=t3[:], accum_out=scr[:], in0=qs[:], in1=u1[:], scale=sc_, bias=sc_
            )
            t4 = wp.tile([12, 512], F32, name=f"t4w{p}_{r}", tag="t4w", bufs=2)
            nc.vector.reciprocal_approx_fast(out=t4[:], in_=t3[:])
            fkt = wp.tile([12, 512], F32, name=f"fkw{p}_{r}", tag="fkw", bufs=4)
            nc.vector.tensor_mul(out=fkt[:].bitcast(F32R), in0=qs[:], in1=t4[:])
            fkt_all[p] = fkt
        vs_all = {}
        for p in grp:
            vs = []
            for kg, (c0, m) in enumerate(KGS):
                vfb = pp.tile([m, 512], F32, name=f"vfb{p}_{r}_{kg}", tag="x", bufs=2)
                MMr(
                    out=vfb[:], lhsT=kindT[:, c0 : c0 + m],
                    rhs=fkt_all[p][:], start=True, stop=True,
                )
                if last:
                    v = wp.tile([m, 512], F32, name=f"v{p}_{r}_{kg}", tag=f"vf{kg}", bufs=1)
                else:
                    v = wp.tile([m, 512], BF16, name=f"v{p}_{r}_{kg}", tag=f"vb{kg}", bufs=4)
                nc.vector.tensor_mul(out=v[:], in0=ssb_all[p][kg][:], in1=vfb[:])
                vs.append(v)
            vs_all[p] = vs
        if last:
            for p in grp:
                for kg, (c0, m) in enumerate(KGS):
                    nc.vector.reduce_sum(
                        out=OUTB[kg][:, p * 2 : p * 2 + 2],
                        in_=vs_all[p][kg][:].rearrange("q (i s) -> q i s", i=2),
                        axis=AX.X,
                    )
            return
        vT_all = {}
        for p in grp:
            for it2 in range(2):
                vtp = pp.tile([128, 576], BF16, name=f"vtp{p}_{r}_{it2}", tag="x", bufs=2)
                for sc in range(2):
                    for kg, (c0, m) in enumerate(KGS):
                        nc.tensor.transpose(
                            out=vtp[:, sc * 288 + c0 :][:, 0:m],
                            in_=vs_all[p][kg][:, it2 * 256 + sc * 128 :][:, 0:128],
                            identity=identb[0:m, 0:m],
                        )
                vT = wp.tile([128, 576], BF16, name=f"vT{p}_{r}_{it2}", tag=f"vT{it2}", bufs=4)
                if it2 == 0:
                    nc.vector.tensor_copy(out=vT[:], in_=vtp[:])
                else:
                    nc.scalar.copy(out=vT[:], in_=vtp[:])
                vT_all[(p, it2)] = vT
        agr_all = {}
        for p in grp:
            agr_all[p] = wp.tile([4, 36], F32, name=f"agr{p}_{r}", tag="agr", bufs=4)
        for p in grp:
            for it2 in range(2):
                for ch in range(2):
                    gps = pp.tile([128, 288], F32, name=f"gw{p}_{r}_{it2}_{ch}", tag="acv", bufs=3)
                    for sc in range(2):
                        MM(
                            out=gps[:],
                            lhsT=PT[p][sc][:, (it2 * 2 + ch) * 128 : (it2 * 2 + ch + 1) * 128],
                            rhs=vT_all[(p, it2)][:, sc * 288 : (sc + 1) * 288],
                            start=(sc == 0),
                            stop=(sc == 1),
                        )
                    ga = wp.tile([128, 288], BF16, name=f"gaw{p}_{r}_{it2}_{ch}", tag="ga", bufs=3)
                    nc.vector.tensor_mul(
                        out=ga[:], in0=wf[:, ch * 288 : (ch + 1) * 288], in1=gps[:]
                    )
                    ap_ = pp.tile([4, 288], F32, name=f"apw{p}_{r}_{it2}_{ch}", tag="acv", bufs=3)
                    MM(out=ap_[:], lhsT=uindb[:], rhs=ga[:], start=True, stop=True)
                    nc.vector.reduce_sum(
                        out=agr_all[p][:, it2 * 18 + ch * 9 : it2 * 18 + ch * 9 + 9],
                        in_=ap_[:].rearrange("q (k c) -> q k c", c=32),
                        axis=AX.X,
                    )
        for p in grp:
            bsl = BT[:, p * 36 : (p + 1) * 36]
            nc.vector.tensor_add(out=bsl, in0=bsl, in1=agr_all[p][:])

    # ---- global softmax + WCS build ----
    def softmax_wcs(rnd):
        eb = wp.tile([4, 18 * bl], F32, name=f"eb{rnd}", tag="eb", bufs=2)
        nc.scalar.activation(out=eb[:], in_=BT[:], func=AF.Exp)
        rs = wp.tile([4, 2 * bl], F32, name=f"rs{rnd}", tag="rs", bufs=2)
        nc.vector.reduce_sum(
            out=rs[:], in_=eb[:].rearrange("q (g k) -> q g k", k=9), axis=AX.X
        )
        rr = wp.tile([4, 2 * bl], F32, name=f"rr{rnd}", tag="rr", bufs=2)
        nc.vector.reciprocal_approx_fast(out=rr[:], in_=rs[:])
        ccb = wp.tile([4, 18 * bl], BF16, name=f"ccb{rnd}", tag="ccb", bufs=2)
        nc.vector.tensor_mul(
            out=ccb[:].rearrange("q (g k) -> q g k", k=9),
            in0=eb[:].rearrange("q (g k) -> q g k", k=9),
            in1=rr[:].unsqueeze(2).to_broadcast([4, 2 * bl, 9]),
        )
        crep = pp.tile([128, 18 * bl], F32, name=f"crep{rnd}", tag="acv", bufs=3)
        MM(out=crep[:], lhsT=u4Tb[:], rhs=ccb[:], start=True, stop=True)
        crsb = wp.tile([128, 18 * bl], BF16, name=f"crsb{rnd}", tag="crsb", bufs=2)
        nc.scalar.copy(out=crsb[:], in_=crep[:])
        CRSB[0] = (rnd, crsb)

    def build_wcs(it):
        rnd, crsb = CRSB[0]
        wcs = []
        for ch in range(2):
            w = pq.tile([128, 288], BF16, name=f"wcs{rnd}_{it}_{ch}", tag=f"wcs{it}_{ch}")
            col = (it * 2 + ch) * 9
            nc.vector.tensor_mul(
                out=w[:].rearrange("q (k c) -> q k c", c=32),
                in0=wf[:, ch * 288 : (ch + 1) * 288].rearrange(
                    "q (k c) -> q k c", c=32
                ),
                in1=crsb[:, col : col + 9].unsqueeze(2).to_broadcast([128, 9, 32]),
            )
            wcs.append(w)
        WCS[it] = wcs

    # ---------------- schedule ----------------
    gather(0)
    if bl > 1:
        gather(1)
    for it in range(bl):
        if it + 2 < bl:
            gather(it + 2)
        a_tp(it)
        a_conv(it)
        if it % 2 == 1:
            p = it // 2
            b_prim(p)
            r_s(p, 0)
            r_v(p, 0)
    softmax_wcs(1)
    round_wave(1)
    softmax_wcs(2)
    round_wave(2)
    for kg, (c0, m) in enumerate(KGS):
        nc.sync.dma_start(
            out=aps["out"][:, c0 : c0 + m].rearrange("b c -> c b"),
            in_=OUTB[kg][:],
        )
    es.close()


def _bf16(x):
    import ml_dtypes

    return np.asarray(x, np.float32).astype(ml_dtypes.bfloat16)


def _pack_consts(inputs):
    conv1_w = np.ascontiguousarray(np.asarray(inputs["conv1_w"], np.float32))
    conv1_b = np.asarray(inputs["conv1_b"], np.float32)
    prim_w = np.ascontiguousarray(np.asarray(inputs["prim_w"], np.float32))
    prim_b = np.asarray(inputs["prim_b"], np.float32)
    W = np.asarray(inputs["W"], np.float32)

    w1g = np.zeros((128, 1152), np.float32)
    for g in range(3):
        for ec in range(4):
            blk = np.zeros((128, 96), np.float32)
            for j in range(3):
                t = 3 * j + g
                # rows: e_local; cols: j*32 + c
                blk[:, j * 32 : (j + 1) * 32] = conv1_w[:, ec * 128 : (ec + 1) * 128, t].T
            w1g[:, (g * 4 + ec) * 96 : (g * 4 + ec + 1) * 96] = blk

    w2 = np.zeros((32, 2304), np.float32)
    for t in range(9):
        w2[:, t * 256 : (t + 1) * 256] = prim_w[:, :, t].T

    wfr = W[0].transpose(0, 2, 1, 3).reshape(U, C, K * C)  # [u, c', (k c)]
    wf = np.zeros((128, 576), np.float32)
    for ch in range(2):
        wf[:, ch * 288 : (ch + 1) * 288] = wfr[ch * 4 : (ch + 1) * 4].reshape(128, 288)
    w9 = wf / 9.0

    b1 = conv1_b.reshape(32, 1).copy()
    b2 = prim_b.reshape(2, 128).T.copy()

    sq8 = np.zeros((128, 16), np.float32)
    for p in range(128):
        sq8[p, p // 32] = 1.0
        sq8[p, 12 + p // 32] = 1.0
    it8 = np.zeros((8, 256), np.float32)
    for p in range(128):
        it8[p // 32, p] = 1.0
        it8[4 + p // 32, 128 + p] = 1.0
    kind = np.zeros((128, 36), np.float32)
    for kg in range(3):
        m = 128 if kg < 2 else 32
        for p in range(m):
            kind[p, kg * 12 + kg * 4 + p // 32] = 1.0
    kindT = np.zeros((12, 288), np.float32)
    for kg in range(3):
        m = 128 if kg < 2 else 32
        for p in range(m):
            kindT[kg * 4 + p // 32, kg * 128 + p] = 1.0
    uind = np.zeros((128, 4), np.float32)
    for p in range(128):
        uind[p, p // 32] = 1.0
    u4T = np.zeros((4, 128), np.float32)
    for p in range(128):
        u4T[p // 32, p] = 1.0

    fbc = np.zeros((128, 2), np.float32)
    fbc[:, 0] = 1e-8
    fbc[:, 1] = 1.0

    ident = np.eye(128, dtype=np.float32)

    return {
        "w1g": _bf16(w1g), "w2b": _bf16(w2), "wf": wf, "w9b": _bf16(w9),
        "sq8": _bf16(sq8), "it8": it8, "kindb": _bf16(kind), "kindT": kindT,
        "uindb": _bf16(uind), "u4Tb": _bf16(u4T), "identf": ident,
        "identb": _bf16(ident), "b1": b1, "b2": b2, "fb": fbc,
    }


_NC_CACHE = {}


def build_nc(bl=BL):
    if bl in _NC_CACHE:
        return _NC_CACHE[bl]
    nc = bacc.Bacc(
        "TRN2", target_bir_lowering=False, debug=False, num_devices=NCORES
    )
    shapes = {
        "x": ([bl, L], I32), "mask": ([bl, L], F32), "emb": ([V, E], BF16),
        "w1g": ([128, 1152], BF16), "w2b": ([32, 2304], BF16),
        "wf": ([128, 576], F32), "w9b": ([128, 576], BF16),
        "sq8": ([128, 16], BF16), "it8": ([8, 256], F32R),
        "kindb": ([128, 36], BF16), "kindT": ([12, 288], F32R),
        "uindb": ([128, 4], BF16), "u4Tb": ([4, 128], BF16),
        "identf": ([128, 128], F32), "identb": ([128, 128], BF16),
        "b1": ([32, 1], F32), "b2": ([128, 2], F32), "fb": ([128, 2], F32),
    }
    aps = {
        name: nc.dram_tensor(name, shp, dt, kind="ExternalInput").ap()
        for name, (shp, dt) in shapes.items()
    }
    aps["out"] = nc.dram_tensor("out", [bl, K * C], F32, kind="ExternalOutput").ap()
    with tile.TileContext(nc) as tc:
        _emit(tc, nc, aps, bl)
    nc.compile()
    _NC_CACHE[bl] = nc
    return nc


def make_in_maps(inputs, bl=BL, ncores=NCORES):
    consts = _pack_consts(inputs)
    x = np.ascontiguousarray(np.asarray(inputs["x"], np.int32).reshape(ncores, bl, L))
    mask = np.ascontiguousarray(
        np.asarray(inputs["attention_mask"], np.float32).reshape(ncores, bl, L)
    )
    emb = _bf16(inputs["emb"])
    return [
        {"x": x[i], "mask": mask[i], "emb": emb, **consts} for i in range(ncores)
    ]


def kernel(x, attention_mask, emb, conv1_w, conv1_b, prim_w, prim_b, W):
    inputs = {
        "x": x, "attention_mask": attention_mask, "emb": emb,
        "conv1_w": conv1_w, "conv1_b": conv1_b,
        "prim_w": prim_w, "prim_b": prim_b, "W": W,
    }
    nc = build_nc(BL)
    in_maps = make_in_maps(inputs)
    res = run_bass_kernel_spmd(nc, in_maps, core_ids=list(range(NCORES)))
    out = np.concatenate(
        [res.results[i]["out"].reshape(BL, K, C) for i in range(NCORES)], axis=0
    )
    return out.astype(np.float32)


# revision 16
# speedup vs baseline: 1.0577x; 1.0577x over previous
"""CapsuleNet Trainium2 kernel, v2.

Data-parallel over batch: 64 items -> 8 cores x 8 items (4 pairs).

Math per item (matches reference):
  e   = emb[x] * mask                      [L=512, E=512]
  h   = relu(conv1d(e.T, k=9, pad=4) + b1) [C=32, L=512]
  p   = conv1d(h, k=9, pad=4, stride=2)+b2 [UC=256, S=256]
  p   = squash(p over C-blocks)
  routing (R=3), b logits S-independent:
    c = softmax_k(b); s[k] = sum_u c[u,k] (W[u,k].T @ p_u)
    v[k] = squash_c(s[k]); b += <W[u,k], p_u.T @ v[k]>
  out = mean_s(v)                          [K=9, C=32]

v2 design vs v1 (508us):
  - bf16 data path for all big matmuls (emb table pre-cast on host).
  - conv1 via taps-on-M: 3 tap-groups {t: t%3==g} x 32ch = M=96 rows,
    psum accumulates over (g,ec) with rhs column shift g; only a 3-way
    j-collapse (cross-quadrant DVE adds) remains. 24 matmuls of N=262
    per item instead of 36 of N=512 (3x fewer PE rows).
  - items processed in PAIRS: routing tiles [m, 512] halve instruction
    counts; shared-lhsT matmuls run at N=512.
  - s/G psums read directly by DVE (no SBUF staging copies).
  - reciprocal_approx_fast + affine_mul_reduce for squash chains.
  - softmax exp batched globally (2 ACT table switches, not 32).
"""

import numpy as np

import concourse.bass as bass
import concourse.tile as tile
from concourse import bacc, mybir
from concourse.bass_utils import run_bass_kernel_spmd

F32 = mybir.dt.float32
F32R = mybir.dt.float32r
BF16 = mybir.dt.bfloat16
I32 = mybir.dt.int32
AF = mybir.ActivationFunctionType
AX = mybir.AxisListType

V, E, L = 50000, 512, 512
B, U, C, K, R = 64, 8, 32, 9, 3
S = 256
NCORES = 8
BL = B // NCORES

KGS = [(0, 128), (128, 128), (256, 32)]


def _emit(tc, nc, aps, bl):
    from contextlib import ExitStack

    es = ExitStack()
    np_pairs = bl // 2

    def MM(out, lhsT, rhs, **kw):
        return nc.tensor.matmul(out=out, lhsT=lhsT, rhs=rhs, **kw)

    def MMr(out, lhsT, rhs, **kw):
        return nc.tensor.matmul(
            out=out, lhsT=lhsT.bitcast(F32R), rhs=rhs.bitcast(F32R), **kw
        )

    cp = es.enter_context(tc.tile_pool(name="consts", bufs=1))

    def const(cname, shape, dt):
        t = cp.tile(shape, dt, name=cname)
        nc.sync.dma_start(out=t[:], in_=aps[cname])
        return t

    w1g = const("w1g", [128, 1152], BF16)
    w2b = const("w2b", [32, 2304], BF16)
    wf = const("wf", [128, 576], F32)
    w9b = const("w9b", [128, 576], BF16)
    sq8 = const("sq8", [128, 16], BF16)
    it8 = const("it8", [8, 256], F32R)
    kindb = const("kindb", [128, 36], BF16)
    kindT = const("kindT", [12, 288], F32R)
    uindb = const("uindb", [128, 4], BF16)
    u4Tb = const("u4Tb", [4, 128], BF16)
    identf = const("identf", [128, 128], F32)
    identb = const("identb", [128, 128], BF16)
    b1 = const("b1", [32, 1], F32)
    b2 = const("b2", [128, 2], F32)
    fb = const("fb", [128, 2], F32)  # col0=1e-8, col1=1.0

    gp = es.enter_context(tc.tile_pool(name="gather", bufs=1))
    wp = es.enter_context(tc.tile_pool(name="work", bufs=2))
    pq = es.enter_context(tc.tile_pool(name="persist", bufs=1))
    pp = es.enter_context(tc.tile_pool(name="psum", bufs=1, space="PSUM"))

    # ---- batched index/mask load ----
    idxs = gp.tile([128, bl * 4], I32, name="idxs")
    nc.sync.dma_start(
        out=idxs[:].rearrange("p (it lc) -> p it lc", lc=4),
        in_=aps["x"].rearrange("it (lc p) -> p it lc", p=128),
    )
    msks = gp.tile([128, bl * 4], F32, name="msks")
    nc.sync.dma_start(
        out=msks[:].rearrange("p (it lc) -> p it lc", lc=4),
        in_=aps["mask"].rearrange("it (lc p) -> p it lc", p=128),
    )

    ERAW = {}

    def gather(it):
        for lc in range(4):
            er = gp.tile([128, 512], BF16, name=f"er{it}_{lc}", tag=f"er{lc}", bufs=3)
            col = it * 4 + lc
            nc.gpsimd.indirect_dma_start(
                out=er[:],
                out_offset=None,
                in_=aps["emb"],
                in_offset=bass.IndirectOffsetOnAxis(ap=idxs[:, col : col + 1], axis=0),
            )
            ERAW[(it, lc)] = er

    ET = [None] * bl
    HP = [None] * np_pairs
    PST = [None] * np_pairs
    PT = [None] * np_pairs
    WCS = [None] * bl
    SPS = {}
    CRSB = [None]
    S2 = [None] * np_pairs
    FKT = [None] * np_pairs
    VSB = [None] * np_pairs

    BT = pq.tile([4, 18 * bl], F32, name="BT")
    OUTB = [
        pq.tile([m, bl], F32, name=f"outb{kg}") for kg, (c0, m) in enumerate(KGS)
    ]

    # ---- stage A1: mask-diag transposes -> eTall ----
    def a_tp(it):
        eT = pq.tile([128, 4 * 528], BF16, name=f"eT{it}")
        ET[it] = eT
        er = eT[:].rearrange("p (ec l) -> p ec l", ec=4)
        nc.vector.memset(er[:, :, 0:4], 0.0)
        nc.vector.memset(er[:, :, 516:520], 0.0)
        tps = [
            pp.tile([128, 1024], BF16, name=f"tp{it}_{half}", tag="acv", bufs=3)
            for half in range(2)
        ]
        for lc in range(4):
            dm = wp.tile([128, 128], BF16, name=f"dm{it}_{lc}", tag="dm", bufs=4)
            col = it * 4 + lc
            nc.vector.tensor_scalar_mul(
                out=dm[:], in0=identf[:], scalar1=msks[:, col : col + 1]
            )
            for ec in range(4):
                nc.tensor.transpose(
                    out=tps[ec // 2][:, (ec % 2) * 512 + lc * 128 :][:, 0:128],
                    in_=ERAW[(it, lc)][:, ec * 128 : (ec + 1) * 128],
                    identity=dm[:],
                )
        for ec in range(4):
            src = tps[ec // 2][:, (ec % 2) * 512 :][:, 0:512]
            dst = eT[:, ec * 528 + 4 : ec * 528 + 516]
            if ec % 2 == 0:
                nc.vector.tensor_copy(out=dst, in_=src)
            else:
                nc.scalar.copy(out=dst, in_=src)

    # ---- stage A2: conv1 (taps-on-M) + collapse + relu ----
    def a_conv(it):
        p, it2 = it // 2, it % 2
        if it2 == 0:
            hp = pq.tile([32, 1056], BF16, name=f"hp{p}")
            HP[p] = hp
            hr = hp[:].rearrange("q (i l) -> q i l", i=2)
            nc.vector.memset(hr[:, :, 0:4], 0.0)
            nc.vector.memset(hr[:, :, 516:520], 0.0)
        hp = HP[p]
        eT = ET[it]
        for h in range(2):
            z = pp.tile([96, 262], F32, name=f"cv{it}_{h}", tag="acv", bufs=3)
            cnt = 0
            for g in range(3):
                for ec in range(4):
                    MM(
                        out=z[:],
                        lhsT=w1g[:, (g * 4 + ec) * 96 : (g * 4 + ec + 1) * 96],
                        rhs=eT[:, ec * 528 + h * 256 + g : ec * 528 + h * 256 + g + 262],
                        start=(cnt == 0),
                        stop=(cnt == 11),
                    )
                    cnt += 1
            zb = wp.tile([32, 256], F32, name=f"zb{it}_{h}", tag="zb", bufs=2)
            nc.scalar.copy(out=zb[:], in_=z[32:64, 3:259])
            u = wp.tile([32, 256], F32, name=f"u{it}_{h}", tag="clps", bufs=2)
            nc.vector.tensor_add(out=u[:], in0=z[0:32, 0:256], in1=zb[:])
            hpre = wp.tile([32, 256], F32, name=f"hpre{it}_{h}", tag="hpre", bufs=2)
            nc.vector.tensor_add(out=hpre[:], in0=u[:], in1=z[64:96, 6:262])
            nc.scalar.activation(
                out=hp[:, it2 * 528 + 4 + h * 256 : it2 * 528 + 4 + h * 256 + 256],
                in_=hpre[:],
                func=AF.Relu,
                bias=b1[:, 0:1],
            )

    # ---- stage B: primary conv (pair), squash-p, ps_t, pT ----
    def b_prim(p):
        hp = HP[p]
        prs = []
        for h in range(2):
            pr = pp.tile([128, 512], F32, name=f"pr{p}_{h}", tag="x", bufs=2)
            for t in range(9):
                rhs = hp[:].rearrange("q (i l) -> q i l", i=2)[:, :, t : t + 512]
                rhs = rhs.rearrange("q i (s two) -> q i s two", two=2)[:, :, :, 0]
                MM(
                    out=pr[:],
                    lhsT=w2b[:, t * 256 + h * 128 : t * 256 + (h + 1) * 128],
                    rhs=rhs,
                    start=(t == 0),
                    stop=(t == 8),
                )
            prs.append(pr)
        psb, p2 = [], []
        for h in range(2):
            sb = wp.tile([128, 512], F32, name=f"psb{p}_{h}", tag=f"psb{h}", bufs=2)
            nc.scalar.activation(
                out=sb[:], in_=prs[h][:], func=AF.Identity, bias=b2[:, h : h + 1]
            )
            psb.append(sb)
            q = wp.tile([128, 512], BF16, name=f"p2{p}_{h}", tag=f"p2{h}", bufs=2)
            nc.scalar.square(out=q[:], in_=prs[h][:])
            p2.append(q)
        psq = pp.tile([8, 512], F32, name=f"psq{p}", tag="x", bufs=2)
        MM(out=psq[:], lhsT=sq8[:, 0:8], rhs=p2[0][:], start=True, stop=False)
        MM(out=psq[:], lhsT=sq8[:, 8:16], rhs=p2[1][:], start=False, stop=True)
        u1 = wp.tile([8, 512], F32, name=f"u1p{p}", tag="u1p", bufs=1)
        nc.scalar.activation(out=u1[:], in_=psq[:], func=AF.Sqrt, bias=fb[0:8, 0:1])
        t3 = wp.tile([8, 512], F32, name=f"t3p{p}", tag="t3p", bufs=1)
        scr = wp.tile([8, 1], F32, name=f"scrp{p}", tag="scrp", bufs=1)
        nc.vector.affine_mul_reduce(
            out=t3[:], accum_out=scr[:], in0=psq[:], in1=u1[:], scale=1.0, bias=1.0
        )
        t4 = wp.tile([8, 512], F32, name=f"t4p{p}", tag="u1p", bufs=1)
        nc.vector.reciprocal_approx_fast(out=t4[:], in_=t3[:])
        f8 = wp.tile([8, 512], F32, name=f"f8{p}", tag="f8", bufs=1)
        nc.vector.tensor_mul(out=f8[:].bitcast(F32R), in0=psq[:], in1=t4[:])
        pst = []
        for h in range(2):
            pfb = pp.tile([128, 512], F32, name=f"pfb{p}_{h}", tag="x", bufs=2)
            MMr(out=pfb[:], lhsT=it8[:, h * 128 : (h + 1) * 128], rhs=f8[:],
                start=True, stop=True)
            ps = pq.tile([128, 512], BF16, name=f"pst{p}_{h}")
            nc.vector.tensor_mul(out=ps[:], in0=psb[h][:], in1=pfb[:])
            pst.append(ps)
        PST[p] = pst
        ptp = pp.tile([128, 1024], BF16, name=f"ptp{p}", tag="x", bufs=2)
        for sc in range(2):
            for it2 in range(2):
                for h in range(2):
                    nc.tensor.transpose(
                        out=ptp[:, sc * 512 + (it2 * 2 + h) * 128 :][:, 0:128],
                        in_=pst[h][:, it2 * 256 + sc * 128 : it2 * 256 + sc * 128 + 128],
                        identity=identb[:],
                    )
        pT = []
        for sc in range(2):
            t = pq.tile([128, 512], BF16, name=f"pT{p}_{sc}")
            if sc == 0:
                nc.vector.tensor_copy(out=t[:], in_=ptp[:, 0:512])
            else:
                nc.scalar.copy(out=t[:], in_=ptp[:, 512:1024])
            pT.append(t)
        PT[p] = pT

    # ---- routing: s + |s|^2 + squash-s ----
    def r_s(p, r):
        pst = PST[p]
        sps = []
        for kg, (c0, m) in enumerate(KGS):
            sp = pp.tile([m, 512], F32, name=f"s{p}_{r}_{kg}", tag="s", bufs=3)
            if r == 0:
                for ch in range(2):
                    MM(
                        out=sp[:],
                        lhsT=w9b[:, ch * 288 + c0 : ch * 288 + c0 + m],
                        rhs=pst[ch][:],
                        start=(ch == 0),
                        stop=(ch == 1),
                    )
            else:
                for it2 in range(2):
                    for ch in range(2):
                        MM(
                            out=sp[:, it2 * 256 : (it2 + 1) * 256],
                            lhsT=WCS[p * 2 + it2][ch][:, c0 : c0 + m],
                            rhs=pst[ch][:, it2 * 256 : (it2 + 1) * 256],
                            start=(ch == 0),
                            stop=(ch == 1),
                        )
            sps.append(sp)
        ssb = []
        for kg, (c0, m) in enumerate(KGS):
            sb = wp.tile([m, 512], BF16, name=f"ssb{p}_{r}_{kg}", tag=f"sb{kg}", bufs=4)
            if kg == 1:
                nc.vector.tensor_copy(out=sb[:], in_=sps[kg][:])
            else:
                nc.scalar.copy(out=sb[:], in_=sps[kg][:])
            ssb.append(sb)
        SPS[p] = ssb
        s2 = []
        for kg, (c0, m) in enumerate(KGS):
            q = wp.tile([m, 512], BF16, name=f"s2_{p}_{r}_{kg}", tag=f"sq{kg}", bufs=4)
            if kg == 1:
                nc.vector.tensor_mul(out=q[:], in0=ssb[kg][:], in1=ssb[kg][:])
            else:
                nc.scalar.square(out=q[:], in_=sps[kg][:])
            s2.append(q)
        S2[p] = s2
        qk = pp.tile([12, 512], F32, name=f"qk{p}_{r}", tag="x", bufs=2)
        for kg, (c0, m) in enumerate(KGS):
            MM(
                out=qk[:],
                lhsT=kindb[0:m, kg * 12 : (kg + 1) * 12],
                rhs=s2[kg][:],
                start=(kg == 0),
                stop=(kg == 2),
            )
        u1 = wp.tile([12, 512], F32, name=f"u1k{p}_{r}", tag="u1k", bufs=1)
        nc.scalar.activation(out=u1[:], in_=qk[:], func=AF.Sqrt, bias=fb[0:12, 0:1])
        sc_ = float(S) if r == R - 1 else 1.0
        t3 = wp.tile([12, 512], F32, name=f"t3k{p}_{r}", tag="t3k", bufs=1)
        scr = wp.tile([12, 1], F32, name=f"scrk{p}_{r}", tag="scrk", bufs=2)
        nc.vector.affine_mul_reduce(
            out=t3[:], accum_out=scr[:], in0=qk[:], in1=u1[:], scale=sc_, bias=sc_
        )
        t4 = wp.tile([12, 512], F32, name=f"t4k{p}_{r}", tag="t4k", bufs=1)
        nc.vector.reciprocal_approx_fast(out=t4[:], in_=t3[:])
        fkt = wp.tile([12, 512], F32, name=f"fkt{p}_{r}", tag="fk", bufs=1)
        nc.vector.tensor_mul(out=fkt[:].bitcast(F32R), in0=qk[:], in1=t4[:])
        FKT[p] = fkt

    # ---- routing: v (+ transpose + agree for r<R-1, emit for last r) ----
    def r_v(p, r):
        sps = SPS[p]
        fkt = FKT[p]
        last = r == R - 1
        vs = []
        for kg, (c0, m) in enumerate(KGS):
            vfb = pp.tile([m, 512], F32, name=f"vfb{p}_{r}_{kg}", tag="x", bufs=2)
            MMr(out=vfb[:], lhsT=kindT[:, c0 : c0 + m], rhs=fkt[:],
                start=True, stop=True)
            if last:
                v = wp.tile([m, 512], F32, name=f"v{p}_{r}_{kg}", tag=f"vf{kg}", bufs=1)
            else:
                v = wp.tile([m, 512], BF16, name=f"v{p}_{r}_{kg}", tag=f"vb{kg}", bufs=4)
            nc.vector.tensor_mul(out=v[:], in0=sps[kg][:], in1=vfb[:])
            vs.append(v)
        VSB[p] = vs
        if last:
            for kg, (c0, m) in enumerate(KGS):
                nc.vector.reduce_sum(
                    out=OUTB[kg][:, p * 2 : p * 2 + 2],
                    in_=vs[kg][:].rearrange("q (i s) -> q i s", i=2),
                    axis=AX.X,
                )
            return
        agr = wp.tile([4, 36], F32, name=f"agr{p}_{r}", tag="agr", bufs=4)
        for it2 in range(2):
            vtp = pp.tile([128, 576], BF16, name=f"vtp{p}_{r}_{it2}", tag="x", bufs=2)
            for sc in range(2):
                for kg, (c0, m) in enumerate(KGS):
                    nc.tensor.transpose(
                        out=vtp[:, sc * 288 + c0 :][:, 0:m],
                        in_=vs[kg][:, it2 * 256 + sc * 128 :][:, 0:128],
                        identity=identb[0:m, 0:m],
                    )
            vT = wp.tile([128, 576], BF16, name=f"vT{p}_{r}_{it2}", tag=f"vT{it2}", bufs=4)
            if it2 == 0:
                nc.vector.tensor_copy(out=vT[:], in_=vtp[:])
            else:
                nc.scalar.copy(out=vT[:], in_=vtp[:])
            for ch in range(2):
                gps = pp.tile([128, 288], F32, name=f"g{p}_{r}_{it2}_{ch}", tag="x", bufs=2)
                for sc in range(2):
                    MM(
                        out=gps[:],
                        lhsT=PT[p][sc][:, (it2 * 2 + ch) * 128 : (it2 * 2 + ch + 1) * 128],
                        rhs=vT[:, sc * 288 : (sc + 1) * 288],
                        start=(sc == 0),
                        stop=(sc == 1),
                    )
                ga = wp.tile([128, 288], BF16, name=f"ga{p}_{r}_{it2}_{ch}", tag="ga", bufs=3)
                nc.vector.tensor_mul(
                    out=ga[:], in0=wf[:, ch * 288 : (ch + 1) * 288], in1=gps[:]
                )
                ap_ = pp.tile([4, 288], F32, name=f"ap{p}_{r}_{it2}_{ch}", tag="x", bufs=2)
                MM(out=ap_[:], lhsT=uindb[:], rhs=ga[:], start=True, stop=True)
                nc.vector.reduce_sum(
                    out=agr[:, it2 * 18 + ch * 9 : it2 * 18 + ch * 9 + 9],
                    in_=ap_[:].rearrange("q (k c) -> q k c", c=32),
                    axis=AX.X,
                )
        bsl = BT[:, p * 36 : (p + 1) * 36]
        if r == 0:
            nc.vector.tensor_copy(out=bsl, in_=agr[:])
        else:
            nc.vector.tensor_add(out=bsl, in0=bsl, in1=agr[:])

    # ---- waved rounds r1/r2: all pairs per stage for engine-level overlap ----
    def round_wave(r):
        for grp in ([0, 1], [2, 3]) if np_pairs == 4 else ([list(range(np_pairs))]):
            round_wave_grp(r, grp)

    def round_wave_grp(r, grp):
        last = r == R - 1
        sps_all = {}
        for p in grp:
            build_wcs(p * 2)
            build_wcs(p * 2 + 1)
            pst = PST[p]
            sps = []
            for kg, (c0, m) in enumerate(KGS):
                sp = pp.tile([m, 512], F32, name=f"s{p}_{r}_{kg}", tag="s", bufs=3)
                for it2 in range(2):
                    for ch in range(2):
                        MM(
                            out=sp[:, it2 * 256 : (it2 + 1) * 256],
                            lhsT=WCS[p * 2 + it2][ch][:, c0 : c0 + m],
                            rhs=pst[ch][:, it2 * 256 : (it2 + 1) * 256],
                            start=(ch == 0),
                            stop=(ch == 1),
                        )
                sps.append(sp)
            sps_all[p] = sps
        ssb_all = {}
        for p in grp:
            ssb = []
            for kg, (c0, m) in enumerate(KGS):
                sb = wp.tile([m, 512], BF16, name=f"ssb{p}_{r}_{kg}", tag=f"sb{kg}", bufs=4)
                if kg == 1:
                    nc.vector.tensor_copy(out=sb[:], in_=sps_all[p][kg][:])
                else:
                    nc.scalar.copy(out=sb[:], in_=sps_all[p][kg][:])
                ssb.append(sb)
            ssb_all[p] = ssb
        s2_all = {}
        for p in grp:
            s2 = []
            for kg, (c0, m) in enumerate(KGS):
                q = wp.tile([m, 512], BF16, name=f"s2_{p}_{r}_{kg}", tag=f"sq{kg}", bufs=4)
                if kg == 1:
                    nc.vector.tensor_mul(out=q[:], in0=ssb_all[p][kg][:], in1=ssb_all[p][kg][:])
                else:
                    nc.scalar.square(out=q[:], in_=sps_all[p][kg][:])
                s2.append(q)
            s2_all[p] = s2
        sc_ = float(S) if last else 1.0
        fkt_all = {}
        for p in grp:
            qk = pp.tile([12, 512], F32, name=f"qk{p}_{r}", tag="x", bufs=2)
            for kg, (c0, m) in enumerate(KGS):
                MM(
                    out=qk[:],
                    lhsT=kindb[0:m, kg * 12 : (kg + 1) * 12],
                    rhs=s2_all[p][kg][:],
                    start=(kg == 0),
                    stop=(kg == 2),
                )
            qs = wp.tile([12, 512], F32, name=f"qs{p}_{r}", tag="qs", bufs=2)
            nc.vector.tensor_copy(out=qs[:], in_=qk[:])
            u1 = wp.tile([12, 512], F32, name=f"u1w{p}_{r}", tag="u1w", bufs=2)
            nc.scalar.activation(out=u1[:], in_=qk[:], func=AF.Sqrt, bias=fb[0:12, 0:1])
            t3 = wp.tile([12, 512], F32, name=f"t3w{p}_{r}", tag="t3w", bufs=2)
            scr = wp.tile([12, 1], F32, name=f"scrw{p}_{r}", tag="scrw", bufs=2)
            nc.vector.affine_mul_reduce(
                out=t3[:], accum_out=scr[:], in0=qs[:], in1=u1[:], scale=sc_, bias=sc_
            )
            t4 = wp.tile([12, 512], F32, name=f"t4w{p}_{r}", tag="t4w", bufs=2)
            nc.vector.reciprocal_approx_fast(out=t4[:], in_=t3[:])
            fkt = wp.tile([12, 512], F32, name=f"fkw{p}_{r}", tag="fkw", bufs=4)
            nc.vector.tensor_mul(out=fkt[:].bitcast(F32R), in0=qs[:], in1=t4[:])
            fkt_all[p] = fkt
        vs_all = {}
        for p in grp:
            vs = []
            for kg, (c0, m) in enumerate(KGS):
                vfb = pp.tile([m, 512], F32, name=f"vfb{p}_{r}_{kg}", tag="x", bufs=2)
                MMr(
                    out=vfb[:], lhsT=kindT[:, c0 : c0 + m],
                    rhs=fkt_all[p][:], start=True, stop=True,
                )
                if last:
                    v = wp.tile([m, 512], F32, name=f"v{p}_{r}_{kg}", tag=f"vf{kg}", bufs=1)
                else:
                    v = wp.tile([m, 512], BF16, name=f"v{p}_{r}_{kg}", tag=f"vb{kg}", bufs=4)
                nc.vector.tensor_mul(out=v[:], in0=ssb_all[p][kg][:], in1=vfb[:])
                vs.append(v)
            vs_all[p] = vs
        if last:
            for p in grp:
                for kg, (c0, m) in enumerate(KGS):
                    nc.vector.reduce_sum(
                        out=OUTB[kg][:, p * 2 : p * 2 + 2],
                        in_=vs_all[p][kg][:].rearrange("q (i s) -> q i s", i=2),
                        axis=AX.X,
                    )
            return
        vT_all = {}
        for p in grp:
            for it2 in range(2):
                vtp = pp.tile([128, 576], BF16, name=f"vtp{p}_{r}_{it2}", tag="x", bufs=2)
                for sc in range(2):
                    for kg, (c0, m) in enumerate(KGS):
                        nc.tensor.transpose(
                            out=vtp[:, sc * 288 + c0 :][:, 0:m],
                            in_=vs_all[p][kg][:, it2 * 256 + sc * 128 :][:, 0:128],
                            identity=identb[0:m, 0:m],
                        )
                vT = wp.tile([128, 576], BF16, name=f"vT{p}_{r}_{it2}", tag=f"vT{it2}", bufs=4)
                if it2 == 0:
                    nc.vector.tensor_copy(out=vT[:], in_=vtp[:])
                else:
                    nc.scalar.copy(out=vT[:], in_=vtp[:])
                vT_all[(p, it2)] = vT
        agr_all = {}
        for p in grp:
            agr_all[p] = wp.tile([4, 36], F32, name=f"agr{p}_{r}", tag="agr", bufs=4)
        for p in grp:
            for it2 in range(2):
                for ch in range(2):
                    gps = pp.tile([128, 288], F32, name=f"gw{p}_{r}_{it2}_{ch}", tag="acv", bufs=3)
                    for sc in range(2):
                        MM(
                            out=gps[:],
                            lhsT=PT[p][sc][:, (it2 * 2 + ch) * 128 : (it2 * 2 + ch + 1) * 128],
                            rhs=vT_all[(p, it2)][:, sc * 288 : (sc + 1) * 288],
                            start=(sc == 0),
                            stop=(sc == 1),
                        )
                    ga = wp.tile([128, 288], BF16, name=f"gaw{p}_{r}_{it2}_{ch}", tag="ga", bufs=3)
                    nc.vector.tensor_mul(
                        out=ga[:], in0=wf[:, ch * 288 : (ch + 1) * 288], in1=gps[:]
                    )
                    ap_ = pp.tile([4, 288], F32, name=f"apw{p}_{r}_{it2}_{ch}", tag="acv", bufs=3)
                    MM(out=ap_[:], lhsT=uindb[:], rhs=ga[:], start=True, stop=True)
                    nc.vector.reduce_sum(
                        out=agr_all[p][:, it2 * 18 + ch * 9 : it2 * 18 + ch * 9 + 9],
                        in_=ap_[:].rearrange("q (k c) -> q k c", c=32),
                        axis=AX.X,
                    )
        for p in grp:
            bsl = BT[:, p * 36 : (p + 1) * 36]
            nc.vector.tensor_add(out=bsl, in0=bsl, in1=agr_all[p][:])

    # ---- global softmax + WCS build ----
    def softmax_wcs(rnd):
        eb = wp.tile([4, 18 * bl], F32, name=f"eb{rnd}", tag="eb", bufs=2)
        nc.scalar.activation(out=eb[:], in_=BT[:], func=AF.Exp)
        rs = wp.tile([4, 2 * bl], F32, name=f"rs{rnd}", tag="rs", bufs=2)
        nc.vector.reduce_sum(
            out=rs[:], in_=eb[:].rearrange("q (g k) -> q g k", k=9), axis=AX.X
        )
        rr = wp.tile([4, 2 * bl], F32, name=f"rr{rnd}", tag="rr", bufs=2)
        nc.vector.reciprocal_approx_fast(out=rr[:], in_=rs[:])
        ccb = wp.tile([4, 18 * bl], BF16, name=f"ccb{rnd}", tag="ccb", bufs=2)
        nc.vector.tensor_mul(
            out=ccb[:].rearrange("q (g k) -> q g k", k=9),
            in0=eb[:].rearrange("q (g k) -> q g k", k=9),
            in1=rr[:].unsqueeze(2).to_broadcast([4, 2 * bl, 9]),
        )
        crep = pp.tile([128, 18 * bl], F32, name=f"crep{rnd}", tag="acv", bufs=3)
        MM(out=crep[:], lhsT=u4Tb[:], rhs=ccb[:], start=True, stop=True)
        crsb = wp.tile([128, 18 * bl], BF16, name=f"crsb{rnd}", tag="crsb", bufs=2)
        nc.scalar.copy(out=crsb[:], in_=crep[:])
        CRSB[0] = (rnd, crsb)

    def build_wcs(it):
        rnd, crsb = CRSB[0]
        wcs = []
        for ch in range(2):
            w = pq.tile([128, 288], BF16, name=f"wcs{rnd}_{it}_{ch}", tag=f"wcs{it}_{ch}")
            col = (it * 2 + ch) * 9
            nc.vector.tensor_mul(
                out=w[:].rearrange("q (k c) -> q k c", c=32),
                in0=wf[:, ch * 288 : (ch + 1) * 288].rearrange(
                    "q (k c) -> q k c", c=32
                ),
                in1=crsb[:, col : col + 9].unsqueeze(2).to_broadcast([128, 9, 32]),
            )
            wcs.append(w)
        WCS[it] = wcs

    # ---------------- schedule ----------------
    gather(0)
    if bl > 1:
        gather(1)
    for it in range(bl):
        if it + 2 < bl:
            gather(it + 2)
        a_tp(it)
        a_conv(it)
        if it % 2 == 1:
            p = it // 2
            b_prim(p)
            r_s(p, 0)
            r_v(p, 0)
    softmax_wcs(1)
    round_wave(1)
    softmax_wcs(2)
    round_wave(2)
    for kg, (c0, m) in enumerate(KGS):
        nc.sync.dma_start(
            out=aps["out"][:, c0 : c0 + m].rearrange("b c -> c b"),
            in_=OUTB[kg][:],
        )
    es.close()


def _bf16(x):
    import ml_dtypes

    return np.asarray(x, np.float32).astype(ml_dtypes.bfloat16)


def _pack_consts(inputs):
    conv1_w = np.ascontiguousarray(np.asarray(inputs["conv1_w"], np.float32))
    conv1_b = np.asarray(inputs["conv1_b"], np.float32)
    prim_w = np.ascontiguousarray(np.asarray(inputs["prim_w"], np.float32))
    prim_b = np.asarray(inputs["prim_b"], np.float32)
    W = np.asarray(inputs["W"], np.float32)

    w1g = np.zeros((128, 1152), np.float32)
    for g in range(3):
        for ec in range(4):
            blk = np.zeros((128, 96), np.float32)
            for j in range(3):
                t = 3 * j + g
                # rows: e_local; cols: j*32 + c
                blk[:, j * 32 : (j + 1) * 32] = conv1_w[:, ec * 128 : (ec + 1) * 128, t].T
            w1g[:, (g * 4 + ec) * 96 : (g * 4 + ec + 1) * 96] = blk

    w2 = np.zeros((32, 2304), np.float32)
    for t in range(9):
        w2[:, t * 256 : (t + 1) * 256] = prim_w[:, :, t].T

    wfr = W[0].transpose(0, 2, 1, 3).reshape(U, C, K * C)  # [u, c', (k c)]
    wf = np.zeros((128, 576), np.float32)
    for ch in range(2):
        wf[:, ch * 288 : (ch + 1) * 288] = wfr[ch * 4 : (ch + 1) * 4].reshape(128, 288)
    w9 = wf / 9.0

    b1 = conv1_b.reshape(32, 1).copy()
    b2 = prim_b.reshape(2, 128).T.copy()

    sq8 = np.zeros((128, 16), np.float32)
    for p in range(128):
        sq8[p, p // 32] = 1.0
        sq8[p, 12 + p // 32] = 1.0
    it8 = np.zeros((8, 256), np.float32)
    for p in range(128):
        it8[p // 32, p] = 1.0
        it8[4 + p // 32, 128 + p] = 1.0
    kind = np.zeros((128, 36), np.float32)
    for kg in range(3):
        m = 128 if kg < 2 else 32
        for p in range(m):
            kind[p, kg * 12 + kg * 4 + p // 32] = 1.0
    kindT = np.zeros((12, 288), np.float32)
    for kg in range(3):
        m = 128 if kg < 2 else 32
        for p in range(m):
            kindT[kg * 4 + p // 32, kg * 128 + p] = 1.0
    uind = np.zeros((128, 4), np.float32)
    for p in range(128):
        uind[p, p // 32] = 1.0
    u4T = np.zeros((4, 128), np.float32)
    for p in range(128):
        u4T[p // 32, p] = 1.0

    fbc = np.zeros((128, 2), np.float32)
    fbc[:, 0] = 1e-8
    fbc[:, 1] = 1.0

    ident = np.eye(128, dtype=np.float32)

    return {
        "w1g": _bf16(w1g), "w2b": _bf16(w2), "wf": wf, "w9b": _bf16(w9),
        "sq8": _bf16(sq8), "it8": it8, "kindb": _bf16(kind), "kindT": kindT,
        "uindb": _bf16(uind), "u4Tb": _bf16(u4T), "identf": ident,
        "identb": _bf16(ident), "b1": b1, "b2": b2, "fb": fbc,
    }


_NC_CACHE = {}


def build_nc(bl=BL):
    if bl in _NC_CACHE:
        return _NC_CACHE[bl]
    nc = bacc.Bacc(
        "TRN2", target_bir_lowering=False, debug=False, num_devices=NCORES
    )
    shapes = {
        "x": ([bl, L], I32), "mask": ([bl, L], F32), "emb": ([V, E], BF16),
        "w1g": ([128, 1152], BF16), "w2b": ([32, 2304], BF16),
        "wf": ([128, 576], F32), "w9b": ([128, 576], BF16),
        "sq8": ([128, 16], BF16), "it8": ([8, 256], F32R),
        "kindb": ([128, 36], BF16), "kindT": ([12, 288], F32R),
        "uindb": ([128, 4], BF16), "u4Tb": ([4, 128], BF16),
        "identf": ([128, 128], F32), "identb": ([128, 128], BF16),
        "b1": ([32, 1], F32), "b2": ([128, 2], F32), "fb": ([128, 2], F32),
    }
    aps = {
        name: nc.dram_tensor(name, shp, dt, kind="ExternalInput").ap()
        for name, (shp, dt) in shapes.items()
    }
    aps["out"] = nc.dram_tensor("out", [bl, K * C], F32, kind="ExternalOutput").ap()
    with tile.TileContext(nc) as tc:
        _emit(tc, nc, aps, bl)
    nc.compile()
    _NC_CACHE[bl] = nc
    return nc


def make_in_maps(inputs, bl=BL, ncores=NCORES):
    consts = _pack_consts(inputs)
    x = np.ascontiguousarray(np.asarray(inputs["x"], np.int32).reshape(ncores, bl, L))
    mask = np.ascontiguousarray(
        np.asarray(inputs["attention_mask"], np.float32).reshape(ncores, bl, L)
    )
    emb = _bf16(inputs["emb"])
    return [
        {"x": x[i], "mask": mask[i], "emb": emb, **consts} for i in range(ncores)
    ]


def kernel(x, attention_mask, emb, conv1_w, conv1_b, prim_w, prim_b, W):
    inputs = {
        "x": x, "attention_mask": attention_mask, "emb": emb,
        "conv1_w": conv1_w, "conv1_b": conv1_b,
        "prim_w": prim_w, "prim_b": prim_b, "W": W,
    }
    nc = build_nc(BL)
    in_maps = make_in_maps(inputs)
    res = run_bass_kernel_spmd(nc, in_maps, core_ids=list(range(NCORES)))
    out = np.concatenate(
        [res.results[i]["out"].reshape(BL, K, C) for i in range(NCORES)], axis=0
    )
    return out.astype(np.float32)
